# revision 4
# baseline (speedup 1.0000x reference)
# CrossEntropyLoss (ignore_index=0, ragged lengths) for logits [16, 513, 32000] f32.
#
# loss = sum_{valid} (log(sum_v exp(x[r, v])) - x[r, tgt_r]) / n_valid
#   valid = (s < lengths[b]) & (tgt != 0), over rows r = (b, s) with s in [0, 512)
#   (positions are output[:, 1:] / trg[:, 1:])
#
# Strategy: the only heavy work is sum_v exp(x) over the valid rows (~0.5 GB
# streamed from HBM).  Host packs just the valid rows (ragged-skip: on average
# half the positions are beyond their sequence length), shards them across the
# 8 NeuronCores, and the device kernel computes per-row sum(exp(x)) with the
# ScalarEngine's fused exp+accumulate while DMA streams at HBM line rate.
# Everything else (target gather, mask, log, final divide) is O(B*S) host work.
#
# Device layout: rows are packed flat, 32 rows per block viewed as [128, 8000]
# (each partition holds a quarter of one row), so every DMA uses all 128 SBUF
# ports and padding granularity is 32*8 = 256 rows.  Per block: one 4 MB DMA,
# two exp+accumulate ACTs (4000 wide) writing per-partition partial sums into
# an accumulator column; one tiny DMA at the end stores all partials.  Host
# adds the 4 quarters (x 2 halves) per row.

import math

import numpy as np

B, SP1, V = 16, 513, 32000
S = SP1 - 1
N_CORES = 8
P = 128
ROWS_PER_BLOCK = 32           # 32 rows -> [128, 8000] flat
BLOCK_F = V * ROWS_PER_BLOCK // P   # 8000 free elems per partition
HALF_F = BLOCK_F // 2         # 4000-wide ACT ops
SPLIT = P // ROWS_PER_BLOCK   # 4 partitions per row

_NC_CACHE: dict = {}


def _build_nc(n_blocks: int, bufs_in: int = 5):
    import concourse.bacc as bacc
    import concourse.mybir as mybir
    import concourse.tile as tile

    key = (n_blocks, bufs_in)
    if key in _NC_CACHE:
        return _NC_CACHE[key]

    nc = bacc.Bacc("TRN2", target_bir_lowering=False, debug=False,
                   num_devices=N_CORES)
    x = nc.dram_tensor("x", [n_blocks, P, BLOCK_F], mybir.dt.float32,
                       kind="ExternalInput").ap()
    out = nc.dram_tensor("out", [P, 2 * n_blocks], mybir.dt.float32,
                         kind="ExternalOutput").ap()

    with tile.TileContext(nc) as tc:
        with (
            tc.tile_pool(name="data", bufs=bufs_in) as dpool,
            tc.tile_pool(name="acc", bufs=1) as apool,
        ):
            acc = apool.tile([P, 2 * n_blocks], mybir.dt.float32)
            for b in range(n_blocks):
                t = dpool.tile([P, BLOCK_F], mybir.dt.float32)
                nc.sync.dma_start(t[:], x[b])
                for h in range(2):
                    sl = t[:, h * HALF_F:(h + 1) * HALF_F]
                    nc.scalar.activation(
                        sl, sl, mybir.ActivationFunctionType.Exp,
                        accum_out=acc[:, 2 * b + h:2 * b + h + 1])
            nc.sync.dma_start(out[:], acc[:])

    nc.compile()
    _NC_CACHE[key] = nc
    return nc


def _run_device(shards: np.ndarray, trace: bool = False):
    """shards: [8, n_blocks, 128, BLOCK_F] f32.  Returns (rowsum
    [8 * n_blocks * 32] float64 per-row sum(exp), exec_time_ns or None)."""
    from concourse.bass_utils import run_bass_kernel_spmd

    n_blocks = shards.shape[1]
    nc = _build_nc(n_blocks)
    in_maps = [{"x": shards[i]} for i in range(N_CORES)]
    res = run_bass_kernel_spmd(nc, in_maps, core_ids=list(range(N_CORES)),
                               trace=trace)
    outs = np.stack([res.results[i]["out"] for i in range(N_CORES)])
    # outs: [8, 128, 2*n_blocks]; column c=(b,h), partition p=(r,q)
    rowsum = (outs.astype(np.float64)
              .transpose(0, 2, 1)                      # [8, 2nb, 128]
              .reshape(N_CORES, n_blocks, 2, ROWS_PER_BLOCK, SPLIT)
              .sum(axis=(2, 4))                        # [8, nb, 32]
              .reshape(-1))
    return rowsum, res.exec_time_ns


def _prepare(output, trg, lengths):
    """Host-side packing: returns (shards [8, n_blocks, 128, BLOCK_F],
    n_valid, sum of gathered target logits) or None if no valid targets."""
    output = np.asarray(output, dtype=np.float32)
    trg = np.asarray(trg)
    lengths = np.asarray(lengths).astype(np.int64)

    tgt = trg[:, 1:]
    pos_valid = np.arange(S)[None, :] < lengths[:, None]
    valid = pos_valid & (tgt != 0)
    n_valid = int(valid.sum())
    if n_valid == 0:
        return None

    rb, rs = np.nonzero(valid)
    flat = output.reshape(B * SP1, V)           # contiguous view, no copy
    row_idx = rb * SP1 + (rs + 1)               # skip BOS position
    tgt_vals = tgt[rb, rs].astype(np.int64)
    x_t_sum = flat[row_idx, tgt_vals].astype(np.float64).sum()

    group = N_CORES * ROWS_PER_BLOCK
    rows_per_core = max(1, math.ceil(n_valid / group)) * ROWS_PER_BLOCK
    n_blocks = rows_per_core // ROWS_PER_BLOCK
    total = rows_per_core * N_CORES
    packed = np.zeros((total, V), dtype=np.float32)
    np.take(flat, row_idx, axis=0, out=packed[:n_valid])
    return packed.reshape(N_CORES, n_blocks, P, BLOCK_F), n_valid, x_t_sum


def kernel(output, trg, lengths):
    prep = _prepare(output, trg, lengths)
    if prep is None:
        return np.array(0.0, dtype=np.float32)
    shards, n_valid, x_t_sum = prep
    rowsum, _ = _run_device(shards)
    log_z = np.log(rowsum[:n_valid])
    loss = (log_z.sum() - x_t_sum) / n_valid
    return np.array(loss, dtype=np.float32)


# revision 6
# speedup vs baseline: 1.0261x; 1.0261x over previous
# CrossEntropyLoss (ignore_index=0, ragged lengths) for logits [16, 513, 32000] f32.
#
# loss = sum_{valid} (log(sum_v exp(x[r, v])) - x[r, tgt_r]) / n_valid
#   valid = (s < lengths[b]) & (tgt != 0), over rows r = (b, s) with s in [0, 512)
#   (positions are output[:, 1:] / trg[:, 1:])
#
# Strategy: the only heavy work is sum_v exp(x) over the valid rows (~0.5 GB
# streamed from HBM).  Host packs just the valid rows (ragged-skip: on average
# half the positions are beyond their sequence length), shards them across the
# 8 NeuronCores, and the device kernel computes per-row sum(exp(x)) with the
# ScalarEngine's fused exp+accumulate while DMA streams at HBM line rate.
# Everything else (target gather, mask, log, final divide) is O(B*S) host work.
#
# Device layout: rows are packed flat, 32 rows per block viewed as [128, 8000]
# (each partition holds a quarter of one row), so every DMA uses all 128 SBUF
# ports and padding granularity is 32*8 = 256 rows.  Per block: one 4 MB DMA,
# two exp+accumulate ACTs (4000 wide) writing per-partition partial sums into
# an accumulator column; one tiny DMA at the end stores all partials.  Host
# adds the 4 quarters (x 2 halves) per row.

import math

import numpy as np

B, SP1, V = 16, 513, 32000
S = SP1 - 1
N_CORES = 8
P = 128
ROWS_PER_BLOCK = 32           # 32 rows -> [128, 8000] flat
BLOCK_F = V * ROWS_PER_BLOCK // P   # 8000 free elems per partition
HALF_F = BLOCK_F // 2         # 4000-wide ACT ops
SPLIT = P // ROWS_PER_BLOCK   # 4 partitions per row

_NC_CACHE: dict = {}


def _build_nc(n_blocks: int, bufs_in: int = 10):
    import concourse.bacc as bacc
    import concourse.mybir as mybir
    import concourse.tile as tile

    key = (n_blocks, bufs_in)
    if key in _NC_CACHE:
        return _NC_CACHE[key]

    nc = bacc.Bacc("TRN2", target_bir_lowering=False, debug=False,
                   num_devices=N_CORES)
    x = nc.dram_tensor("x", [n_blocks, P, BLOCK_F], mybir.dt.float32,
                       kind="ExternalInput").ap()
    out = nc.dram_tensor("out", [P, 2 * n_blocks], mybir.dt.float32,
                         kind="ExternalOutput").ap()

    with tile.TileContext(nc) as tc:
        with (
            tc.tile_pool(name="data", bufs=bufs_in) as dpool,
            tc.tile_pool(name="acc", bufs=1) as apool,
        ):
            acc = apool.tile([P, 2 * n_blocks], mybir.dt.float32)
            for b in range(n_blocks):
                for h in range(2):
                    # 16000B partition lines: SDMA engines sustain line rate
                    # at this packet size (32000B lines measured ~15% slower)
                    t = dpool.tile([P, HALF_F], mybir.dt.float32)
                    nc.sync.dma_start(
                        t[:], x[b, :, h * HALF_F:(h + 1) * HALF_F])
                    nc.scalar.activation(
                        t[:], t[:], mybir.ActivationFunctionType.Exp,
                        accum_out=acc[:, 2 * b + h:2 * b + h + 1])
            nc.sync.dma_start(out[:], acc[:])

    nc.compile()
    _NC_CACHE[key] = nc
    return nc


def _run_device(shards: np.ndarray, trace: bool = False):
    """shards: [8, n_blocks, 128, BLOCK_F] f32.  Returns (rowsum
    [8 * n_blocks * 32] float64 per-row sum(exp), exec_time_ns or None)."""
    from concourse.bass_utils import run_bass_kernel_spmd

    n_blocks = shards.shape[1]
    nc = _build_nc(n_blocks)
    in_maps = [{"x": shards[i]} for i in range(N_CORES)]
    res = run_bass_kernel_spmd(nc, in_maps, core_ids=list(range(N_CORES)),
                               trace=trace)
    outs = np.stack([res.results[i]["out"] for i in range(N_CORES)])
    # outs: [8, 128, 2*n_blocks]; column c=(b,h), partition p=(r,q)
    rowsum = (outs.astype(np.float64)
              .transpose(0, 2, 1)                      # [8, 2nb, 128]
              .reshape(N_CORES, n_blocks, 2, ROWS_PER_BLOCK, SPLIT)
              .sum(axis=(2, 4))                        # [8, nb, 32]
              .reshape(-1))
    return rowsum, res.exec_time_ns


def _prepare(output, trg, lengths):
    """Host-side packing: returns (shards [8, n_blocks, 128, BLOCK_F],
    n_valid, sum of gathered target logits) or None if no valid targets."""
    output = np.asarray(output, dtype=np.float32)
    trg = np.asarray(trg)
    lengths = np.asarray(lengths).astype(np.int64)

    tgt = trg[:, 1:]
    pos_valid = np.arange(S)[None, :] < lengths[:, None]
    valid = pos_valid & (tgt != 0)
    n_valid = int(valid.sum())
    if n_valid == 0:
        return None

    rb, rs = np.nonzero(valid)
    flat = output.reshape(B * SP1, V)           # contiguous view, no copy
    row_idx = rb * SP1 + (rs + 1)               # skip BOS position
    tgt_vals = tgt[rb, rs].astype(np.int64)
    x_t_sum = flat[row_idx, tgt_vals].astype(np.float64).sum()

    group = N_CORES * ROWS_PER_BLOCK
    rows_per_core = max(1, math.ceil(n_valid / group)) * ROWS_PER_BLOCK
    n_blocks = rows_per_core // ROWS_PER_BLOCK
    total = rows_per_core * N_CORES
    packed = np.zeros((total, V), dtype=np.float32)
    np.take(flat, row_idx, axis=0, out=packed[:n_valid])
    return packed.reshape(N_CORES, n_blocks, P, BLOCK_F), n_valid, x_t_sum


def kernel(output, trg, lengths):
    prep = _prepare(output, trg, lengths)
    if prep is None:
        return np.array(0.0, dtype=np.float32)
    shards, n_valid, x_t_sum = prep
    rowsum, _ = _run_device(shards)
    log_z = np.log(rowsum[:n_valid])
    loss = (log_z.sum() - x_t_sum) / n_valid
    return np.array(loss, dtype=np.float32)


# revision 9
# speedup vs baseline: 1.2691x; 1.2369x over previous
# CrossEntropyLoss (ignore_index=0, ragged lengths) for logits [16, 513, 32000] f32.
#
# loss = sum_{valid} (log(sum_v exp(x[r, v])) - x[r, tgt_r]) / n_valid
#   valid = (s < lengths[b]) & (tgt != 0), over rows r = (b, s) with s in [0, 512)
#   (positions are output[:, 1:] / trg[:, 1:])
#
# Strategy: the only heavy work is sum_v exp(x) over the valid rows (~0.5 GB
# streamed from HBM).  Host packs just the valid rows (ragged-skip: on average
# half the positions are beyond their sequence length), shards them across the
# 8 NeuronCores, and the device kernel computes per-row sum(exp(x)) with the
# ScalarEngine's fused exp+accumulate while DMA streams at HBM line rate.
# Everything else (target gather, mask, log, final divide) is O(B*S) host work.
#
# Device layout: rows are packed flat, 32 rows per block viewed as [128, 8000]
# (each partition holds a quarter of one row), so every DMA uses all 128 SBUF
# ports and padding granularity is 32*8 = 256 rows.  Per block: one 4 MB DMA,
# two exp+accumulate ACTs (4000 wide) writing per-partition partial sums into
# an accumulator column; one tiny DMA at the end stores all partials.  Host
# adds the 4 quarters (x 2 halves) per row.

import math

import numpy as np

B, SP1, V = 16, 513, 32000
S = SP1 - 1
N_CORES = 8
P = 128
ROW_F = V // P                # 250: free elems per partition for ONE row
CHUNK_ROWS = 16               # 16 rows -> one [128, 4000] DMA/ACT chunk
CHUNK_F = ROW_F * CHUNK_ROWS  # 4000 (16000B partition lines: line-rate DMA)
TAIL_ROWS = 8                 # tail granularity: [128, 2000] chunks
TAIL_F = ROW_F * TAIL_ROWS

_NC_CACHE: dict = {}


def _chunk_plan(rows_per_core: int):
    """List of chunk sizes (in rows) covering rows_per_core."""
    n_main, rem = divmod(rows_per_core, CHUNK_ROWS)
    return [CHUNK_ROWS] * n_main + [TAIL_ROWS] * (rem // TAIL_ROWS)


def _build_nc(rows_per_core: int, bufs_in: int = 10):
    import concourse.bacc as bacc
    import concourse.mybir as mybir
    import concourse.tile as tile

    key = (rows_per_core, bufs_in)
    if key in _NC_CACHE:
        return _NC_CACHE[key]

    plan = _chunk_plan(rows_per_core)
    n_cols = len(plan)
    total_f = rows_per_core * ROW_F

    nc = bacc.Bacc("TRN2", target_bir_lowering=False, debug=False,
                   num_devices=N_CORES)
    assert total_f * P == rows_per_core * V
    x = nc.dram_tensor("x", [rows_per_core * V], mybir.dt.float32,
                       kind="ExternalInput").ap()
    out = nc.dram_tensor("out", [P, n_cols], mybir.dt.float32,
                         kind="ExternalOutput").ap()

    with tile.TileContext(nc) as tc:
        with (
            tc.tile_pool(name="data", bufs=bufs_in) as dpool,
            tc.tile_pool(name="acc", bufs=1) as apool,
        ):
            acc = apool.tile([P, n_cols], mybir.dt.float32)
            off = 0
            for c, rows in enumerate(plan):
                f = rows * ROW_F
                src = x[off:off + P * f].rearrange("(p f) -> p f", p=P)
                t = dpool.tile([P, f], mybir.dt.float32)
                nc.sync.dma_start(t[:], src)
                nc.scalar.activation(
                    t[:], t[:], mybir.ActivationFunctionType.Exp,
                    accum_out=acc[:, c:c + 1])
                off += P * f
            nc.sync.dma_start(out[:], acc[:])

    nc.compile()
    _NC_CACHE[key] = nc
    return nc


def _run_device(shards: np.ndarray, trace: bool = False, trace_cores=None):
    """shards: [8, rows_per_core * V] f32 flat per core.  Returns (rowsum
    [8 * rows_per_core] float64 per-row sum(exp), exec_time_ns or None)."""
    from concourse.bass_utils import run_bass_kernel_spmd

    rows_per_core = shards.shape[1] // V
    plan = _chunk_plan(rows_per_core)
    nc = _build_nc(rows_per_core)
    in_maps = [{"x": shards[i]} for i in range(N_CORES)]
    kw = {}
    if trace_cores is not None:
        kw["trace_cores"] = trace_cores
    res = run_bass_kernel_spmd(nc, in_maps, core_ids=list(range(N_CORES)),
                               trace=trace, **kw)
    outs = np.stack([res.results[i]["out"] for i in range(N_CORES)])
    # outs: [8, 128, n_cols]; chunk c covers `plan[c]` rows; within chunk c,
    # partition p holds 1/(P/rows) of row  r = p // (P // rows_c).
    rowsum = np.empty((N_CORES, rows_per_core), dtype=np.float64)
    r0 = 0
    for c, rows in enumerate(plan):
        split = P // rows
        col = outs[:, :, c].astype(np.float64)       # [8, 128]
        rowsum[:, r0:r0 + rows] = col.reshape(N_CORES, rows, split).sum(-1)
        r0 += rows
    return rowsum.reshape(-1), res.exec_time_ns


def _prepare(output, trg, lengths):
    """Host-side packing: returns (shards [8, n_blocks, 128, BLOCK_F],
    n_valid, sum of gathered target logits) or None if no valid targets."""
    output = np.asarray(output, dtype=np.float32)
    trg = np.asarray(trg)
    lengths = np.asarray(lengths).astype(np.int64)

    tgt = trg[:, 1:]
    pos_valid = np.arange(S)[None, :] < lengths[:, None]
    valid = pos_valid & (tgt != 0)
    n_valid = int(valid.sum())
    if n_valid == 0:
        return None

    rb, rs = np.nonzero(valid)
    flat = output.reshape(B * SP1, V)           # contiguous view, no copy
    row_idx = rb * SP1 + (rs + 1)               # skip BOS position
    tgt_vals = tgt[rb, rs].astype(np.int64)
    x_t_sum = flat[row_idx, tgt_vals].astype(np.float64).sum()

    group = N_CORES * TAIL_ROWS
    rows_per_core = max(1, math.ceil(n_valid / group)) * TAIL_ROWS
    total = rows_per_core * N_CORES
    packed = np.zeros((total, V), dtype=np.float32)
    np.take(flat, row_idx, axis=0, out=packed[:n_valid])
    return packed.reshape(N_CORES, rows_per_core * V), n_valid, x_t_sum


def kernel(output, trg, lengths):
    prep = _prepare(output, trg, lengths)
    if prep is None:
        return np.array(0.0, dtype=np.float32)
    shards, n_valid, x_t_sum = prep
    rowsum, _ = _run_device(shards)
    log_z = np.log(rowsum[:n_valid])
    loss = (log_z.sum() - x_t_sum) / n_valid
    return np.array(loss, dtype=np.float32)
